# revision 2
# baseline (speedup 1.0000x reference)
"""Trainium2 Bass kernel for a small dense MLP head over a wide input.

Network (all f32, affine until the final sigmoid):
    level      = state @ W1 + b1                      # [B, 50]
    combined   = concat([level, hidden], axis=1)      # [B, 100]
    level2     = combined @ W2 + b2                   # [B, 32]
    hidden_new = level2 @ Wh + bh                     # [B, 50]
    output     = sigmoid(hidden_new @ Wo + bo)        # [B, 1]

Strategy: pure data parallel over the batch (8192 rows -> 1024 per core),
weights replicated.  Each shard's activations are passed transposed
([features, batch]) so the contraction dim sits on SBUF partitions — the
weights as stored ([in, out]) are then exactly the PE's lhsT layout and no
on-device transposes are needed anywhere.  The dominant cost is streaming
state (16.8 MB/core); it is DMA'd in 2 MB contiguous slabs viewed as
[128, 4096], which row-permutes K — harmless because W1's rows are permuted
identically (the same natural reshape).  The concat never materializes:
level2 = level @ W2[:50] + hidden @ W2[50:] as two accumulating matmuls.
"""

import sys

for _p in ("/opt/trn_rl_repo",):
    if _p not in sys.path:
        sys.path.insert(0, _p)

from contextlib import ExitStack

import numpy as np

import concourse.bass as bass
import concourse.tile as tile
from concourse import bacc, mybir
from concourse.bass_utils import run_bass_kernel_spmd

N_CORES = 8
BATCH = 8192
B = BATCH // N_CORES  # 1024 rows per core
K = 4096              # state features (contraction dim of the big matmul)
H1 = 50               # level / hidden width
H2 = 32               # level2 width
HALF = 512            # fp32 matmul max moving free dim; also one PSUM bank
SLAB_ROWS = 512       # K-rows per DMA slab -> [128, 4096] = 2 MB per dma_start
N_SLAB = K // SLAB_ROWS          # 8
ROWS_PER_PART = SLAB_ROWS // 128  # 4 K-rows interleaved per partition

F32 = mybir.dt.float32

_CACHE = {}


def _build_nc():
    """Build + compile the per-core Bass program (identical on all cores)."""
    nc = bacc.Bacc("TRN2", target_bir_lowering=False, debug=False)

    # ---- DRAM parameters (per-core shard views, pre-laid-out by the host) ----
    stateT_d = nc.dram_tensor("stateT", [N_SLAB, 128, ROWS_PER_PART * B], F32,
                              kind="ExternalInput")
    w1p_d = nc.dram_tensor("W1p", [128, N_SLAB * ROWS_PER_PART * H1], F32,
                           kind="ExternalInput")
    hiddenT_d = nc.dram_tensor("hiddenT", [H1, B], F32, kind="ExternalInput")
    b1_d = nc.dram_tensor("b1", [H1, 1], F32, kind="ExternalInput")
    w2a_d = nc.dram_tensor("W2a", [H1, H2], F32, kind="ExternalInput")
    w2b_d = nc.dram_tensor("W2b", [H1, H2], F32, kind="ExternalInput")
    b2_d = nc.dram_tensor("b2", [H2, 1], F32, kind="ExternalInput")
    wh_d = nc.dram_tensor("Wh", [H2, H1], F32, kind="ExternalInput")
    bh_d = nc.dram_tensor("bh", [H1, 1], F32, kind="ExternalInput")
    wo_d = nc.dram_tensor("Wo", [H1, 1], F32, kind="ExternalInput")
    bo_d = nc.dram_tensor("bo", [1, 1], F32, kind="ExternalInput")
    hnT_d = nc.dram_tensor("hnT", [H1, B], F32, kind="ExternalOutput")
    outT_d = nc.dram_tensor("outT", [1, B], F32, kind="ExternalOutput")

    Ident = mybir.ActivationFunctionType.Identity
    Sigm = mybir.ActivationFunctionType.Sigmoid

    with tile.TileContext(nc) as tc, ExitStack() as ctx:
        consts = ctx.enter_context(tc.tile_pool(name="consts", bufs=1))
        stp = ctx.enter_context(tc.tile_pool(name="state", bufs=4))
        work = ctx.enter_context(tc.tile_pool(name="work", bufs=2))
        psum = ctx.enter_context(
            tc.tile_pool(name="psum", bufs=1, space=bass.MemorySpace.PSUM))

        # ---- constants: weights, biases, the hidden shard ----
        w1v = consts.tile([128, N_SLAB * ROWS_PER_PART * H1], F32, tag="w1")
        nc.sync.dma_start(w1v[:], w1p_d[:])
        hidT = consts.tile([H1, B], F32, tag="hid")
        nc.sync.dma_start(hidT[:], hiddenT_d[:])
        w2a = consts.tile([H1, H2], F32, tag="w2a")
        nc.sync.dma_start(w2a[:], w2a_d[:])
        w2b = consts.tile([H1, H2], F32, tag="w2b")
        nc.sync.dma_start(w2b[:], w2b_d[:])
        whs = consts.tile([H2, H1], F32, tag="wh")
        nc.sync.dma_start(whs[:], wh_d[:])
        wos = consts.tile([H1, 1], F32, tag="wo")
        nc.sync.dma_start(wos[:], wo_d[:])
        b1s = consts.tile([H1, 1], F32, tag="b1")
        nc.sync.dma_start(b1s[:], b1_d[:])
        b2s = consts.tile([H2, 1], F32, tag="b2")
        nc.sync.dma_start(b2s[:], b2_d[:])
        bhs = consts.tile([H1, 1], F32, tag="bh")
        nc.sync.dma_start(bhs[:], bh_d[:])
        bos = consts.tile([1, 1], F32, tag="bo")
        nc.sync.dma_start(bos[:], bo_d[:])

        # ---- main accumulation: levelT[h] += W1_chunk.T @ stateT_chunk ----
        lv = [psum.tile([H1, HALF], F32, tag=f"lv{h}", name=f"lv{h}")
              for h in range(B // HALF)]
        n_j = ROWS_PER_PART
        for s in range(N_SLAB):
            st = stp.tile([128, n_j * B], F32, tag="st")
            nc.sync.dma_start(st[:], stateT_d[s])
            for j in range(n_j):
                w1c = w1v[:, (s * n_j + j) * H1:(s * n_j + j + 1) * H1]
                for h in range(B // HALF):
                    nc.tensor.matmul(
                        lv[h][:],
                        w1c,
                        st[:, j * B + h * HALF: j * B + h * HALF + HALF],
                        start=(s == 0 and j == 0),
                        stop=(s == N_SLAB - 1 and j == n_j - 1),
                    )

        # ---- epilogue per batch half: 3 tiny matmuls + biases + sigmoid ----
        for h in range(B // HALF):
            cols = slice(h * HALF, (h + 1) * HALF)
            lvs = work.tile([H1, HALF], F32, tag="lvs")
            nc.scalar.activation(lvs[:], lv[h][:], Ident, bias=b1s[:])

            l2p = psum.tile([H2, HALF], F32, tag="l2")
            nc.tensor.matmul(l2p[:], w2a[:], lvs[:], start=True, stop=False)
            nc.tensor.matmul(l2p[:], w2b[:], hidT[:, cols], start=False, stop=True)
            l2s = work.tile([H2, HALF], F32, tag="l2s")
            nc.scalar.activation(l2s[:], l2p[:], Ident, bias=b2s[:])

            hnp = psum.tile([H1, HALF], F32, tag="hn")
            nc.tensor.matmul(hnp[:], whs[:], l2s[:], start=True, stop=True)
            hns = work.tile([H1, HALF], F32, tag="hns")
            nc.scalar.activation(hns[:], hnp[:], Ident, bias=bhs[:])
            nc.sync.dma_start(hnT_d[:, cols], hns[:])

            op = psum.tile([1, HALF], F32, tag="op")
            nc.tensor.matmul(op[:], wos[:], hns[:], start=True, stop=True)
            osb = work.tile([1, HALF], F32, tag="osb")
            nc.scalar.activation(osb[:], op[:], Sigm, bias=bos[:])
            nc.sync.dma_start(outT_d[:, cols], osb[:])

    nc.compile()
    return nc


def get_nc():
    if "nc" not in _CACHE:
        _CACHE["nc"] = _build_nc()
    return _CACHE["nc"]


def shard_inputs(state, hidden, W1, b1, W2, b2, Wh, bh, Wo, bo):
    """Full inputs -> one input dict per core, in the device layout."""
    f = lambda x: np.ascontiguousarray(np.asarray(x, dtype=np.float32))
    state, hidden = f(state), f(hidden)
    W1, W2, Wh = f(W1), f(W2), f(Wh)
    # W1 rows permuted exactly like the state slab reshape: slab s holds K-rows
    # [512s, 512s+512), partition p within a slab holds rows 4p..4p+3.
    w1p = np.ascontiguousarray(
        W1.reshape(N_SLAB, 128, ROWS_PER_PART * H1).transpose(1, 0, 2)
        .reshape(128, N_SLAB * ROWS_PER_PART * H1))
    common = {
        "W1p": w1p,
        "W2a": np.ascontiguousarray(W2[:H1]),
        "W2b": np.ascontiguousarray(W2[H1:]),
        "Wh": Wh,
        "b1": f(b1).reshape(H1, 1),
        "b2": f(b2).reshape(H2, 1),
        "bh": f(bh).reshape(H1, 1),
        "Wo": f(Wo).reshape(H1, 1),
        "bo": f(bo).reshape(1, 1),
    }
    in_maps = []
    for c in range(N_CORES):
        rows = slice(c * B, (c + 1) * B)
        stateT = np.ascontiguousarray(state[rows].T)  # [K, B]
        in_maps.append({
            "stateT": stateT.reshape(N_SLAB, 128, ROWS_PER_PART * B),
            "hiddenT": np.ascontiguousarray(hidden[rows].T),
            **common,
        })
    return in_maps


def unshard_outputs(results):
    hidden_new = np.concatenate([r["hnT"].T for r in results], axis=0)
    output = np.concatenate([r["outT"].reshape(B, 1) for r in results], axis=0)
    return output.astype(np.float32), hidden_new.astype(np.float32)


def run(in_maps, trace=False):
    res = run_bass_kernel_spmd(get_nc(), in_maps, list(range(N_CORES)),
                               trace=trace)
    _CACHE["last_results"] = res
    return res


def kernel(state, hidden, W1, b1, W2, b2, Wh, bh, Wo, bo):
    in_maps = shard_inputs(state, hidden, W1, b1, W2, b2, Wh, bh, Wo, bo)
    res = run(in_maps, trace=False)
    return unshard_outputs(res.results)


# revision 3
# speedup vs baseline: 1.1385x; 1.1385x over previous
"""Trainium2 Bass kernel for a small dense MLP head over a wide input.

Network (all f32, affine until the final sigmoid):
    level      = state @ W1 + b1                      # [B, 50]
    combined   = concat([level, hidden], axis=1)      # [B, 100]
    level2     = combined @ W2 + b2                   # [B, 32]
    hidden_new = level2 @ Wh + bh                     # [B, 50]
    output     = sigmoid(hidden_new @ Wo + bo)        # [B, 1]

Strategy: pure data parallel over the batch (8192 rows -> 1024 per core),
weights replicated.  Each shard's activations are passed transposed
([features, batch]) so the contraction dim sits on SBUF partitions — the
weights as stored ([in, out]) are then exactly the PE's lhsT layout and no
on-device transposes are needed anywhere.  The concat never materializes:
level2 = level @ W2[:50] + hidden @ W2[50:] as two accumulating matmuls.

The dominant cost is streaming state (16.8 MB/core).  The PE's native fp32
matmul lowers to 2 slow HW passes (~3x the bf16 column rate), so state and
W1 are sent as an exact-split bf16 pair (hi = bf16(x), lo = bf16(x - hi))
— the same 4 bytes/element of traffic, but the product is computed as
hi@W1h + hi@W1l + lo@W1h in three fast bf16 matmuls (the dropped lo@W1l
term is ~2^-16 relative).  State is DMA'd in 1 MB contiguous slabs viewed
as [128, 4096], which row-permutes K — harmless because W1's rows are
permuted identically (the same natural reshape).
"""

import sys

for _p in ("/opt/trn_rl_repo",):
    if _p not in sys.path:
        sys.path.insert(0, _p)

from contextlib import ExitStack

import ml_dtypes
import numpy as np

import concourse.bass as bass
import concourse.tile as tile
from concourse import bacc, mybir
from concourse.bass_utils import run_bass_kernel_spmd

N_CORES = 8
BATCH = 8192
B = BATCH // N_CORES  # 1024 rows per core
K = 4096              # state features (contraction dim of the big matmul)
H1 = 50               # level / hidden width
H2 = 32               # level2 width
HALF = 512            # one PSUM bank of f32; max fp32 moving free dim
SLAB_ROWS = 512       # K-rows per DMA slab -> [128, 4096] bf16 = 1 MB each
N_SLAB = K // SLAB_ROWS           # 8
ROWS_PER_PART = SLAB_ROWS // 128  # 4 K-rows interleaved per partition

F32 = mybir.dt.float32
BF16 = mybir.dt.bfloat16
BF16_NP = ml_dtypes.bfloat16

_CACHE = {}


def _build_nc():
    """Build + compile the per-core Bass program (identical on all cores)."""
    nc = bacc.Bacc("TRN2", target_bir_lowering=False, debug=False)

    # ---- DRAM parameters (per-core shard views, pre-laid-out by the host) ----
    sthi_d = nc.dram_tensor("state_hi", [N_SLAB, 128, ROWS_PER_PART * B], BF16,
                            kind="ExternalInput")
    stlo_d = nc.dram_tensor("state_lo", [N_SLAB, 128, ROWS_PER_PART * B], BF16,
                            kind="ExternalInput")
    w1hi_d = nc.dram_tensor("W1hi", [128, N_SLAB * ROWS_PER_PART * H1], BF16,
                            kind="ExternalInput")
    w1lo_d = nc.dram_tensor("W1lo", [128, N_SLAB * ROWS_PER_PART * H1], BF16,
                            kind="ExternalInput")
    hiddenT_d = nc.dram_tensor("hiddenT", [H1, B], F32, kind="ExternalInput")
    b1_d = nc.dram_tensor("b1", [H1, 1], F32, kind="ExternalInput")
    w2a_d = nc.dram_tensor("W2a", [H1, H2], F32, kind="ExternalInput")
    w2b_d = nc.dram_tensor("W2b", [H1, H2], F32, kind="ExternalInput")
    b2_d = nc.dram_tensor("b2", [H2, 1], F32, kind="ExternalInput")
    wh_d = nc.dram_tensor("Wh", [H2, H1], F32, kind="ExternalInput")
    bh_d = nc.dram_tensor("bh", [H1, 1], F32, kind="ExternalInput")
    wo_d = nc.dram_tensor("Wo", [H1, 1], F32, kind="ExternalInput")
    bo_d = nc.dram_tensor("bo", [1, 1], F32, kind="ExternalInput")
    hnT_d = nc.dram_tensor("hnT", [H1, B], F32, kind="ExternalOutput")
    outT_d = nc.dram_tensor("outT", [1, B], F32, kind="ExternalOutput")

    Ident = mybir.ActivationFunctionType.Identity
    Sigm = mybir.ActivationFunctionType.Sigmoid

    with tile.TileContext(nc) as tc, ExitStack() as ctx:
        consts = ctx.enter_context(tc.tile_pool(name="consts", bufs=1))
        stp = ctx.enter_context(tc.tile_pool(name="state", bufs=4))
        work = ctx.enter_context(tc.tile_pool(name="work", bufs=2))
        psum = ctx.enter_context(
            tc.tile_pool(name="psum", bufs=1, space=bass.MemorySpace.PSUM))

        # ---- constants on the SWDGE queue so the HWDGE ring is state-only ----
        w1hi = consts.tile([128, N_SLAB * ROWS_PER_PART * H1], BF16, tag="w1hi")
        nc.gpsimd.dma_start(w1hi[:], w1hi_d[:])
        w1lo = consts.tile([128, N_SLAB * ROWS_PER_PART * H1], BF16, tag="w1lo")
        nc.gpsimd.dma_start(w1lo[:], w1lo_d[:])
        hidT = consts.tile([H1, B], F32, tag="hid")
        nc.gpsimd.dma_start(hidT[:], hiddenT_d[:])
        w2a = consts.tile([H1, H2], F32, tag="w2a")
        nc.gpsimd.dma_start(w2a[:], w2a_d[:])
        w2b = consts.tile([H1, H2], F32, tag="w2b")
        nc.gpsimd.dma_start(w2b[:], w2b_d[:])
        whs = consts.tile([H2, H1], F32, tag="wh")
        nc.gpsimd.dma_start(whs[:], wh_d[:])
        wos = consts.tile([H1, 1], F32, tag="wo")
        nc.gpsimd.dma_start(wos[:], wo_d[:])
        b1s = consts.tile([H1, 1], F32, tag="b1")
        nc.gpsimd.dma_start(b1s[:], b1_d[:])
        b2s = consts.tile([H2, 1], F32, tag="b2")
        nc.gpsimd.dma_start(b2s[:], b2_d[:])
        bhs = consts.tile([H1, 1], F32, tag="bh")
        nc.gpsimd.dma_start(bhs[:], bh_d[:])
        bos = consts.tile([1, 1], F32, tag="bo")
        nc.gpsimd.dma_start(bos[:], bo_d[:])

        # Warm the ACT sigmoid table under the DMA shadow instead of in the
        # epilogue tail (~1.3 us table load on first use of a new func).
        sigwarm = consts.tile([1, 1], F32, tag="sigwarm")
        nc.scalar.activation(sigwarm[:], bos[:], Sigm, bias=bos[:])

        # ---- main accumulation over K:  levelT[h] in PSUM [50, 512] ----
        lv = [psum.tile([H1, HALF], F32, tag=f"lv{h}", name=f"lv{h}")
              for h in range(B // HALF)]
        n_j = ROWS_PER_PART
        last = (N_SLAB - 1, n_j - 1, 2)
        for s in range(N_SLAB):
            sthi = stp.tile([128, n_j * B], BF16, tag="sthi")
            nc.sync.dma_start(sthi[:], sthi_d[s])
            stlo = stp.tile([128, n_j * B], BF16, tag="stlo")
            nc.sync.dma_start(stlo[:], stlo_d[s])
            for j in range(n_j):
                c = slice((s * n_j + j) * H1, (s * n_j + j + 1) * H1)
                # grouped by stationary: W1hi x {hi, lo}, then W1lo x hi
                for t, (wt, st) in enumerate(
                        ((w1hi, sthi), (w1hi, stlo), (w1lo, sthi))):
                    for h in range(B // HALF):
                        nc.tensor.matmul(
                            lv[h][:],
                            wt[:, c],
                            st[:, j * B + h * HALF: j * B + h * HALF + HALF],
                            start=(s == 0 and j == 0 and t == 0),
                            stop=((s, j, t) == last),
                        )

        # ---- epilogue per batch half: 3 tiny matmuls + biases + sigmoid ----
        hns_tiles = []
        for h in range(B // HALF):
            cols = slice(h * HALF, (h + 1) * HALF)
            lvs = work.tile([H1, HALF], F32, tag="lvs")
            nc.scalar.activation(lvs[:], lv[h][:], Ident, bias=b1s[:])

            l2p = psum.tile([H2, HALF], F32, tag="l2")
            nc.tensor.matmul(l2p[:], w2a[:], lvs[:], start=True, stop=False)
            nc.tensor.matmul(l2p[:], w2b[:], hidT[:, cols], start=False, stop=True)
            l2s = work.tile([H2, HALF], F32, tag="l2s")
            nc.scalar.activation(l2s[:], l2p[:], Ident, bias=b2s[:])

            hnp = psum.tile([H1, HALF], F32, tag="hn")
            nc.tensor.matmul(hnp[:], whs[:], l2s[:], start=True, stop=True)
            hns = work.tile([H1, HALF], F32, tag="hns", name=f"hns{h}")
            nc.scalar.activation(hns[:], hnp[:], Ident, bias=bhs[:])
            nc.sync.dma_start(hnT_d[:, cols], hns[:])
            hns_tiles.append(hns)

        for h, hns in enumerate(hns_tiles):
            cols = slice(h * HALF, (h + 1) * HALF)
            op = psum.tile([1, HALF], F32, tag="op", name=f"op{h}")
            nc.tensor.matmul(op[:], wos[:], hns[:], start=True, stop=True)
            osb = work.tile([1, HALF], F32, tag="osb", name=f"osb{h}")
            nc.scalar.activation(osb[:], op[:], Sigm, bias=bos[:])
            nc.sync.dma_start(outT_d[:, cols], osb[:])

    nc.compile()
    return nc


def get_nc():
    if "nc" not in _CACHE:
        _CACHE["nc"] = _build_nc()
    return _CACHE["nc"]


def _hilo(x):
    hi = x.astype(BF16_NP)
    lo = (x - hi.astype(np.float32)).astype(BF16_NP)
    return hi, lo


def shard_inputs(state, hidden, W1, b1, W2, b2, Wh, bh, Wo, bo):
    """Full inputs -> one input dict per core, in the device layout."""
    f = lambda x: np.ascontiguousarray(np.asarray(x, dtype=np.float32))
    state, hidden = f(state), f(hidden)
    W1, W2, Wh = f(W1), f(W2), f(Wh)
    # W1 rows permuted exactly like the state slab reshape: slab s holds K-rows
    # [512s, 512s+512), partition p within a slab holds rows 4p..4p+3.
    w1p = np.ascontiguousarray(
        W1.reshape(N_SLAB, 128, ROWS_PER_PART * H1).transpose(1, 0, 2)
        .reshape(128, N_SLAB * ROWS_PER_PART * H1))
    w1hi, w1lo = _hilo(w1p)
    common = {
        "W1hi": w1hi,
        "W1lo": w1lo,
        "W2a": np.ascontiguousarray(W2[:H1]),
        "W2b": np.ascontiguousarray(W2[H1:]),
        "Wh": Wh,
        "b1": f(b1).reshape(H1, 1),
        "b2": f(b2).reshape(H2, 1),
        "bh": f(bh).reshape(H1, 1),
        "Wo": f(Wo).reshape(H1, 1),
        "bo": f(bo).reshape(1, 1),
    }
    in_maps = []
    for c in range(N_CORES):
        rows = slice(c * B, (c + 1) * B)
        stateT = np.ascontiguousarray(state[rows].T)  # [K, B]
        st_hi, st_lo = _hilo(stateT)
        shape = (N_SLAB, 128, ROWS_PER_PART * B)
        in_maps.append({
            "state_hi": st_hi.reshape(shape),
            "state_lo": st_lo.reshape(shape),
            "hiddenT": np.ascontiguousarray(hidden[rows].T),
            **common,
        })
    return in_maps


def unshard_outputs(results):
    hidden_new = np.concatenate([r["hnT"].T for r in results], axis=0)
    output = np.concatenate([r["outT"].reshape(B, 1) for r in results], axis=0)
    return output.astype(np.float32), hidden_new.astype(np.float32)


def run(in_maps, trace=False):
    res = run_bass_kernel_spmd(get_nc(), in_maps, list(range(N_CORES)),
                               trace=trace)
    _CACHE["last_results"] = res
    return res


def kernel(state, hidden, W1, b1, W2, b2, Wh, bh, Wo, bo):
    in_maps = shard_inputs(state, hidden, W1, b1, W2, b2, Wh, bh, Wo, bo)
    res = run(in_maps, trace=False)
    return unshard_outputs(res.results)


# revision 4
# speedup vs baseline: 1.1777x; 1.0344x over previous
"""Trainium2 Bass kernel for a small dense MLP head over a wide input.

Network (all f32, affine until the final sigmoid):
    level      = state @ W1 + b1                      # [B, 50]
    combined   = concat([level, hidden], axis=1)      # [B, 100]
    level2     = combined @ W2 + b2                   # [B, 32]
    hidden_new = level2 @ Wh + bh                     # [B, 50]
    output     = sigmoid(hidden_new @ Wo + bo)        # [B, 1]

Strategy: pure data parallel over the batch (8192 rows -> 1024 per core),
weights replicated.  Each shard's activations are passed transposed
([features, batch]) so the contraction dim sits on SBUF partitions — the
weights as stored ([in, out]) are then exactly the PE's lhsT layout and no
on-device transposes are needed anywhere.

The dominant cost is streaming state (16.8 MB/core).  The PE's native fp32
matmul lowers to 2 slow HW passes, so state and W1 are sent as an
exact-split bf16 pair (hi = bf16(x), lo = bf16(x - hi)) — the same
4 bytes/element of traffic.  All four product terms are computed in just
two bf16 matmuls per tile by packing [W1hi | W1lo] as one 100-column
stationary: the MM against state_hi yields hi@hi (psum partitions 0-49)
and lo@hi (50-99); the MM against state_lo adds hi@lo and lo@lo.  The sum
of the two partition halves is the exact product; it is never explicitly
merged — the next layer uses a duplicated stationary [W2a; W2a] and bias
[b1; 0] instead.  The concat with `hidden` also never materializes
(level2 = level @ W2[:50] + hidden @ W2[50:] as accumulating matmuls).

State is DMA'd in 1 MB contiguous slabs viewed as [128, 4096], which
row-permutes K — harmless because W1's rows are permuted identically (the
same natural reshape).  hi slabs ride the Sync HWDGE ring, lo slabs the
Scalar HWDGE ring, small constants the GpSimd SWDGE ring, so the two
big streams run on both hardware DGE rings in parallel.
"""

import sys

for _p in ("/opt/trn_rl_repo",):
    if _p not in sys.path:
        sys.path.insert(0, _p)

from contextlib import ExitStack

import ml_dtypes
import numpy as np

import concourse.bass as bass
import concourse.tile as tile
from concourse import bacc, mybir
from concourse.bass_utils import run_bass_kernel_spmd

N_CORES = 8
BATCH = 8192
B = BATCH // N_CORES  # 1024 rows per core
K = 4096              # state features (contraction dim of the big matmul)
H1 = 50               # level / hidden width
H2 = 32               # level2 width
HALF = 512            # one PSUM bank of f32; max moving free dim
SLAB_ROWS = 512       # K-rows per DMA slab -> [128, 4096] bf16 = 1 MB each
N_SLAB = K // SLAB_ROWS           # 8
ROWS_PER_PART = SLAB_ROWS // 128  # 4 K-rows interleaved per partition
N_CHUNK = N_SLAB * ROWS_PER_PART  # 32 K-chunks of 128

F32 = mybir.dt.float32
BF16 = mybir.dt.bfloat16
BF16_NP = ml_dtypes.bfloat16

_CACHE = {}


def _build_nc():
    """Build + compile the per-core Bass program (identical on all cores)."""
    nc = bacc.Bacc("TRN2", target_bir_lowering=False, debug=False)

    # ---- DRAM parameters (per-core shard views, pre-laid-out by the host) ----
    sthi_d = nc.dram_tensor("state_hi", [N_SLAB, 128, ROWS_PER_PART * B], BF16,
                            kind="ExternalInput")
    stlo_d = nc.dram_tensor("state_lo", [N_SLAB, 128, ROWS_PER_PART * B], BF16,
                            kind="ExternalInput")
    w1cat_d = nc.dram_tensor("W1cat", [128, N_CHUNK * 2 * H1], BF16,
                             kind="ExternalInput")
    hiddenT_d = nc.dram_tensor("hiddenT", [H1, B], F32, kind="ExternalInput")
    b1e_d = nc.dram_tensor("b1ext", [2 * H1, 1], F32, kind="ExternalInput")
    w2aa_d = nc.dram_tensor("W2aa", [2 * H1, H2], F32, kind="ExternalInput")
    w2b_d = nc.dram_tensor("W2b", [H1, H2], F32, kind="ExternalInput")
    b2_d = nc.dram_tensor("b2", [H2, 1], F32, kind="ExternalInput")
    wh_d = nc.dram_tensor("Wh", [H2, H1], F32, kind="ExternalInput")
    bh_d = nc.dram_tensor("bh", [H1, 1], F32, kind="ExternalInput")
    wo_d = nc.dram_tensor("Wo", [H1, 1], F32, kind="ExternalInput")
    bo_d = nc.dram_tensor("bo", [1, 1], F32, kind="ExternalInput")
    hnT_d = nc.dram_tensor("hnT", [H1, B], F32, kind="ExternalOutput")
    outT_d = nc.dram_tensor("outT", [1, B], F32, kind="ExternalOutput")

    Ident = mybir.ActivationFunctionType.Identity
    Sigm = mybir.ActivationFunctionType.Sigmoid

    with tile.TileContext(nc) as tc, ExitStack() as ctx:
        consts = ctx.enter_context(tc.tile_pool(name="consts", bufs=1))
        stp = ctx.enter_context(tc.tile_pool(name="state", bufs=6))
        work = ctx.enter_context(tc.tile_pool(name="work", bufs=2))
        psum = ctx.enter_context(
            tc.tile_pool(name="psum", bufs=1, space=bass.MemorySpace.PSUM))

        # W1 leads the Scalar HWDGE ring (ahead of the state_lo slabs);
        # everything small goes via the GpSimd SWDGE ring.
        w1cat = consts.tile([128, N_CHUNK * 2 * H1], BF16, tag="w1cat")
        nc.scalar.dma_start(w1cat[:], w1cat_d[:])
        hidT = consts.tile([H1, B], F32, tag="hid")
        nc.gpsimd.dma_start(hidT[:], hiddenT_d[:])
        w2aa = consts.tile([2 * H1, H2], F32, tag="w2aa")
        nc.gpsimd.dma_start(w2aa[:], w2aa_d[:])
        w2b = consts.tile([H1, H2], F32, tag="w2b")
        nc.gpsimd.dma_start(w2b[:], w2b_d[:])
        whs = consts.tile([H2, H1], F32, tag="wh")
        nc.gpsimd.dma_start(whs[:], wh_d[:])
        wos = consts.tile([H1, 1], F32, tag="wo")
        nc.gpsimd.dma_start(wos[:], wo_d[:])
        b1e = consts.tile([2 * H1, 1], F32, tag="b1e")
        nc.gpsimd.dma_start(b1e[:], b1e_d[:])
        b2s = consts.tile([H2, 1], F32, tag="b2")
        nc.gpsimd.dma_start(b2s[:], b2_d[:])
        bhs = consts.tile([H1, 1], F32, tag="bh")
        nc.gpsimd.dma_start(bhs[:], bh_d[:])
        bos = consts.tile([1, 1], F32, tag="bo")
        nc.gpsimd.dma_start(bos[:], bo_d[:])

        # Warm the ACT sigmoid table under the DMA shadow instead of in the
        # epilogue tail (~1.3 us table load on first use of a new func).
        sigwarm = consts.tile([1, 1], F32, tag="sigwarm")
        nc.scalar.activation(sigwarm[:], bos[:], Sigm, bias=bos[:])

        # ---- main accumulation over K ----
        # lv[h] psum [100, 512]: partitions 0-49 accumulate W1hi-terms,
        # 50-99 accumulate W1lo-terms; their sum is the exact f32 product.
        lv = [psum.tile([2 * H1, HALF], F32, tag=f"lv{h}", name=f"lv{h}")
              for h in range(B // HALF)]
        n_j = ROWS_PER_PART
        for s in range(N_SLAB):
            sthi = stp.tile([128, n_j * B], BF16, tag="sthi")
            nc.sync.dma_start(sthi[:], sthi_d[s])
            stlo = stp.tile([128, n_j * B], BF16, tag="stlo")
            nc.scalar.dma_start(stlo[:], stlo_d[s])
            for j in range(n_j):
                ch = s * n_j + j
                wc = w1cat[:, ch * 2 * H1:(ch + 1) * 2 * H1]
                for st, t in ((sthi, 0), (stlo, 1)):
                    for h in range(B // HALF):
                        nc.tensor.matmul(
                            lv[h][:],
                            wc,
                            st[:, j * B + h * HALF: j * B + h * HALF + HALF],
                            start=(s == 0 and j == 0 and t == 0),
                            stop=(s == N_SLAB - 1 and j == n_j - 1 and t == 1),
                        )

        # ---- epilogue per batch half: 3 tiny matmuls + biases + sigmoid ----
        hns_tiles = []
        for h in range(B // HALF):
            cols = slice(h * HALF, (h + 1) * HALF)
            lvs = work.tile([2 * H1, HALF], F32, tag="lvs")
            nc.scalar.activation(lvs[:], lv[h][:], Ident, bias=b1e[:])

            l2p = psum.tile([H2, HALF], F32, tag="l2")
            nc.tensor.matmul(l2p[:], w2aa[:], lvs[:], start=True, stop=False)
            nc.tensor.matmul(l2p[:], w2b[:], hidT[:, cols], start=False, stop=True)
            l2s = work.tile([H2, HALF], F32, tag="l2s")
            nc.scalar.activation(l2s[:], l2p[:], Ident, bias=b2s[:])

            hnp = psum.tile([H1, HALF], F32, tag="hn")
            nc.tensor.matmul(hnp[:], whs[:], l2s[:], start=True, stop=True)
            hns = work.tile([H1, HALF], F32, tag="hns", name=f"hns{h}")
            nc.scalar.activation(hns[:], hnp[:], Ident, bias=bhs[:])
            nc.sync.dma_start(hnT_d[:, cols], hns[:])
            hns_tiles.append(hns)

        for h, hns in enumerate(hns_tiles):
            cols = slice(h * HALF, (h + 1) * HALF)
            op = psum.tile([1, HALF], F32, tag="op", name=f"op{h}")
            nc.tensor.matmul(op[:], wos[:], hns[:], start=True, stop=True)
            osb = work.tile([1, HALF], F32, tag="osb", name=f"osb{h}")
            nc.scalar.activation(osb[:], op[:], Sigm, bias=bos[:])
            nc.sync.dma_start(outT_d[:, cols], osb[:])

    nc.compile()
    return nc


def get_nc():
    if "nc" not in _CACHE:
        _CACHE["nc"] = _build_nc()
    return _CACHE["nc"]


def _hilo(x):
    hi = x.astype(BF16_NP)
    lo = (x - hi.astype(np.float32)).astype(BF16_NP)
    return hi, lo


def shard_inputs(state, hidden, W1, b1, W2, b2, Wh, bh, Wo, bo):
    """Full inputs -> one input dict per core, in the device layout."""
    f = lambda x: np.ascontiguousarray(np.asarray(x, dtype=np.float32))
    state, hidden = f(state), f(hidden)
    W1, W2, Wh = f(W1), f(W2), f(Wh)
    b1 = f(b1).reshape(H1, 1)
    # W1 rows permuted exactly like the state slab reshape: slab s holds K-rows
    # [512s, 512s+512), partition p within a slab holds rows 4p..4p+3.
    # Chunk ch covers K-rows with partition p -> row 128*ch' ... ; each chunk's
    # stationary is [hi | lo] side by side (100 columns).
    w1p = (W1.reshape(N_SLAB, 128, ROWS_PER_PART, H1).transpose(1, 0, 2, 3)
           .reshape(128, N_CHUNK, H1))
    w1hi, w1lo = _hilo(w1p)
    w1cat = np.ascontiguousarray(
        np.concatenate([w1hi, w1lo], axis=2).reshape(128, N_CHUNK * 2 * H1))
    common = {
        "W1cat": w1cat,
        "W2aa": np.ascontiguousarray(np.vstack([W2[:H1], W2[:H1]])),
        "W2b": np.ascontiguousarray(W2[H1:]),
        "Wh": Wh,
        "b1ext": np.vstack([b1, np.zeros((H1, 1), np.float32)]),
        "b2": f(b2).reshape(H2, 1),
        "bh": f(bh).reshape(H1, 1),
        "Wo": f(Wo).reshape(H1, 1),
        "bo": f(bo).reshape(1, 1),
    }
    in_maps = []
    for c in range(N_CORES):
        rows = slice(c * B, (c + 1) * B)
        stateT = np.ascontiguousarray(state[rows].T)  # [K, B]
        st_hi, st_lo = _hilo(stateT)
        shape = (N_SLAB, 128, ROWS_PER_PART * B)
        in_maps.append({
            "state_hi": st_hi.reshape(shape),
            "state_lo": st_lo.reshape(shape),
            "hiddenT": np.ascontiguousarray(hidden[rows].T),
            **common,
        })
    return in_maps


def unshard_outputs(results):
    hidden_new = np.concatenate([r["hnT"].T for r in results], axis=0)
    output = np.concatenate([r["outT"].reshape(B, 1) for r in results], axis=0)
    return output.astype(np.float32), hidden_new.astype(np.float32)


def run(in_maps, trace=False):
    res = run_bass_kernel_spmd(get_nc(), in_maps, list(range(N_CORES)),
                               trace=trace)
    _CACHE["last_results"] = res
    return res


def kernel(state, hidden, W1, b1, W2, b2, Wh, bh, Wo, bo):
    in_maps = shard_inputs(state, hidden, W1, b1, W2, b2, Wh, bh, Wo, bo)
    res = run(in_maps, trace=False)
    return unshard_outputs(res.results)


# revision 8
# speedup vs baseline: 1.1909x; 1.0113x over previous
"""Trainium2 Bass kernel for a small dense MLP head over a wide input.

Network (all f32, affine until the final sigmoid):
    level      = state @ W1 + b1                      # [B, 50]
    combined   = concat([level, hidden], axis=1)      # [B, 100]
    level2     = combined @ W2 + b2                   # [B, 32]
    hidden_new = level2 @ Wh + bh                     # [B, 50]
    output     = sigmoid(hidden_new @ Wo + bo)        # [B, 1]

Strategy: pure data parallel over the batch (8192 rows -> 1024 per core),
weights replicated.  Each shard's activations are passed transposed
([features, batch]) so the contraction dim sits on SBUF partitions — the
weights as stored ([in, out]) are then exactly the PE's lhsT layout and no
on-device transposes are needed anywhere.

The dominant cost is streaming state (16.8 MB/core).  The PE's native fp32
matmul lowers to 2 slow HW passes, so state and W1 are sent as an
exact-split bf16 pair (hi = bf16(x), lo = bf16(x - hi)) — the same
4 bytes/element of traffic.  All four product terms are computed in just
two bf16 matmuls per tile by packing [W1hi | W1lo] as one 100-column
stationary: the MM against state_hi yields hi@hi (psum partitions 0-49)
and lo@hi (50-99); the MM against state_lo adds hi@lo and lo@lo.  The sum
of the two partition halves is the exact product; it is never explicitly
merged — the next layer uses a duplicated stationary [W2a; W2a] and bias
[b1; 0] instead.  The concat with `hidden` also never materializes
(level2 = level @ W2[:50] + hidden @ W2[50:] as accumulating matmuls).

State is DMA'd in 1 MB contiguous slabs viewed as [128, 4096], which
row-permutes K — harmless because W1's rows are permuted identically (the
same natural reshape).  hi slabs ride the Sync HWDGE ring, lo slabs the
Scalar HWDGE ring, small constants the GpSimd SWDGE ring, so the two
big streams run on both hardware DGE rings in parallel.
"""

import sys

for _p in ("/opt/trn_rl_repo",):
    if _p not in sys.path:
        sys.path.insert(0, _p)

from contextlib import ExitStack

import ml_dtypes
import numpy as np

import concourse.bass as bass
import concourse.tile as tile
from concourse import bacc, mybir
from concourse.bass_utils import run_bass_kernel_spmd

N_CORES = 8
BATCH = 8192
B = BATCH // N_CORES  # 1024 rows per core
K = 4096              # state features (contraction dim of the big matmul)
H1 = 50               # level / hidden width
H2 = 32               # level2 width
HALF = 512            # one PSUM bank of f32; max moving free dim
SLAB_ROWS = 512       # K-rows per DMA slab -> [128, 4096] bf16 = 1 MB each
N_SLAB = K // SLAB_ROWS           # 8
ROWS_PER_PART = SLAB_ROWS // 128  # 4 K-rows interleaved per partition
N_CHUNK = N_SLAB * ROWS_PER_PART  # 32 K-chunks of 128

F32 = mybir.dt.float32
BF16 = mybir.dt.bfloat16
BF16_NP = ml_dtypes.bfloat16

_CACHE = {}


def _build_nc():
    """Build + compile the per-core Bass program (identical on all cores)."""
    nc = bacc.Bacc("TRN2", target_bir_lowering=False, debug=False)

    # ---- DRAM parameters (per-core shard views, pre-laid-out by the host) ----
    sthi_d = nc.dram_tensor("state_hi", [N_SLAB, 128, ROWS_PER_PART * B], BF16,
                            kind="ExternalInput")
    stlo_d = nc.dram_tensor("state_lo", [N_SLAB, 128, ROWS_PER_PART * B], BF16,
                            kind="ExternalInput")
    w1cat_d = nc.dram_tensor("W1cat", [128, N_CHUNK * 2 * H1], BF16,
                             kind="ExternalInput")
    hiddenT_d = nc.dram_tensor("hiddenT", [H1, B], F32, kind="ExternalInput")
    # Host-fused tail weights: W2ha = [W2a@Wh; W2a@Wh], W2hb = W2b@Wh,
    # bb = b1@W2a@Wh + b2@Wh + bh  (the W2->Wh chain is affine, so it folds).
    w2ha_d = nc.dram_tensor("W2ha", [2 * H1, H1], F32, kind="ExternalInput")
    w2hb_d = nc.dram_tensor("W2hb", [H1, H1], F32, kind="ExternalInput")
    bb_d = nc.dram_tensor("bb", [H1, 1], F32, kind="ExternalInput")
    wo_d = nc.dram_tensor("Wo", [H1, 1], F32, kind="ExternalInput")
    bo_d = nc.dram_tensor("bo", [1, 1], F32, kind="ExternalInput")
    hnT_d = nc.dram_tensor("hnT", [H1, B], F32, kind="ExternalOutput")
    outT_d = nc.dram_tensor("outT", [1, B], F32, kind="ExternalOutput")

    Ident = mybir.ActivationFunctionType.Identity
    Sigm = mybir.ActivationFunctionType.Sigmoid

    with tile.TileContext(nc) as tc, ExitStack() as ctx:
        consts = ctx.enter_context(tc.tile_pool(name="consts", bufs=1))
        stp = ctx.enter_context(tc.tile_pool(name="state", bufs=6))
        work = ctx.enter_context(tc.tile_pool(name="work", bufs=2))
        psum = ctx.enter_context(
            tc.tile_pool(name="psum", bufs=1, space=bass.MemorySpace.PSUM))

        # W1 leads the Scalar HWDGE ring (ahead of the state_lo slabs);
        # everything small goes via the GpSimd SWDGE ring.
        w1cat = consts.tile([128, N_CHUNK * 2 * H1], BF16, tag="w1cat")
        nc.scalar.dma_start(w1cat[:], w1cat_d[:])
        hidT = consts.tile([H1, B], F32, tag="hid")
        nc.gpsimd.dma_start(hidT[:], hiddenT_d[:])
        w2ha = consts.tile([2 * H1, H1], F32, tag="w2ha")
        nc.gpsimd.dma_start(w2ha[:], w2ha_d[:])
        w2hb = consts.tile([H1, H1], F32, tag="w2hb")
        nc.gpsimd.dma_start(w2hb[:], w2hb_d[:])
        wos = consts.tile([H1, 1], F32, tag="wo")
        nc.gpsimd.dma_start(wos[:], wo_d[:])
        bbs = consts.tile([H1, 1], F32, tag="bb")
        nc.gpsimd.dma_start(bbs[:], bb_d[:])
        bos = consts.tile([1, 1], F32, tag="bo")
        nc.gpsimd.dma_start(bos[:], bo_d[:])

        # Warm the ACT sigmoid table under the DMA shadow instead of in the
        # epilogue tail (~1.3 us table load on first use of a new func).
        sigwarm = consts.tile([1, 1], F32, tag="sigwarm")
        nc.scalar.activation(sigwarm[:], bos[:], Sigm, bias=bos[:])

        # ---- main accumulation over K ----
        # lv[h] psum [100, 512]: partitions 0-49 accumulate W1hi-terms,
        # 50-99 accumulate W1lo-terms; their sum is the exact f32 product.
        lv = [psum.tile([2 * H1, HALF], F32, tag=f"lv{h}", name=f"lv{h}")
              for h in range(B // HALF)]
        n_j = ROWS_PER_PART
        for s in range(N_SLAB):
            sthi = stp.tile([128, n_j * B], BF16, tag="sthi")
            nc.sync.dma_start(sthi[:], sthi_d[s])
            stlo = stp.tile([128, n_j * B], BF16, tag="stlo")
            nc.scalar.dma_start(stlo[:], stlo_d[s])
            for j in range(n_j):
                ch = s * n_j + j
                wc = w1cat[:, ch * 2 * H1:(ch + 1) * 2 * H1]
                for st, t in ((sthi, 0), (stlo, 1)):
                    for h in range(B // HALF):
                        nc.tensor.matmul(
                            lv[h][:],
                            wc,
                            st[:, j * B + h * HALF: j * B + h * HALF + HALF],
                            start=(s == 0 and j == 0 and t == 0),
                            stop=(s == N_SLAB - 1 and j == n_j - 1 and t == 1),
                        )

        # ---- epilogue per batch half: 2 fused matmuls + bias, then out head ----
        hns_tiles = []
        for h in range(B // HALF):
            cols = slice(h * HALF, (h + 1) * HALF)
            # pure PSUM->SBUF copy on the otherwise-idle DVE (bias folded
            # into bb host-side, so no ACT needed here)
            lvs = work.tile([2 * H1, HALF], F32, tag="lvs")
            nc.vector.tensor_copy(lvs[:], lv[h][:])

            hnp = psum.tile([H1, HALF], F32, tag="hn")
            nc.tensor.matmul(hnp[:], w2ha[:], lvs[:], start=True, stop=False)
            nc.tensor.matmul(hnp[:], w2hb[:], hidT[:, cols], start=False,
                             stop=True)
            hns = work.tile([H1, HALF], F32, tag="hns", name=f"hns{h}")
            nc.scalar.activation(hns[:], hnp[:], Ident, bias=bbs[:])
            nc.sync.dma_start(hnT_d[:, cols], hns[:])
            hns_tiles.append(hns)

        for h, hns in enumerate(hns_tiles):
            cols = slice(h * HALF, (h + 1) * HALF)
            op = psum.tile([1, HALF], F32, tag="op", name=f"op{h}")
            nc.tensor.matmul(op[:], wos[:], hns[:], start=True, stop=True)
            osb = work.tile([1, HALF], F32, tag="osb", name=f"osb{h}")
            nc.scalar.activation(osb[:], op[:], Sigm, bias=bos[:])
            nc.sync.dma_start(outT_d[:, cols], osb[:])

    nc.compile()
    return nc


def get_nc():
    if "nc" not in _CACHE:
        _CACHE["nc"] = _build_nc()
    return _CACHE["nc"]


def _hilo(x):
    hi = x.astype(BF16_NP)
    lo = (x - hi.astype(np.float32)).astype(BF16_NP)
    return hi, lo


def shard_inputs(state, hidden, W1, b1, W2, b2, Wh, bh, Wo, bo):
    """Full inputs -> one input dict per core, in the device layout."""
    f = lambda x: np.ascontiguousarray(np.asarray(x, dtype=np.float32))
    state, hidden = f(state), f(hidden)
    W1, W2, Wh = f(W1), f(W2), f(Wh)
    b1 = f(b1).reshape(H1, 1)
    # W1 rows permuted exactly like the state slab reshape: slab s holds K-rows
    # [512s, 512s+512), partition p within a slab holds rows 4p..4p+3.
    # Chunk ch covers K-rows with partition p -> row 128*ch' ... ; each chunk's
    # stationary is [hi | lo] side by side (100 columns).
    w1p = (W1.reshape(N_SLAB, 128, ROWS_PER_PART, H1).transpose(1, 0, 2, 3)
           .reshape(128, N_CHUNK, H1))
    w1hi, w1lo = _hilo(w1p)
    w1cat = np.ascontiguousarray(
        np.concatenate([w1hi, w1lo], axis=2).reshape(128, N_CHUNK * 2 * H1))
    # Fuse the affine W2 -> Wh chain (f64 on host for clean rounding):
    #   hidden_new = level @ (W2a@Wh) + hidden @ (W2b@Wh) + (b1@W2a@Wh + b2@Wh + bh)
    W2a64, W2b64 = np.float64(W2[:H1]), np.float64(W2[H1:])
    Wh64 = np.float64(Wh)
    W2ha1 = (W2a64 @ Wh64).astype(np.float32)  # [50, 50]
    W2hb = (W2b64 @ Wh64).astype(np.float32)   # [50, 50]
    bb = (np.float64(b1[:, 0]) @ W2a64 @ Wh64 + np.float64(f(b2)) @ Wh64
          + np.float64(f(bh))).astype(np.float32)
    common = {
        "W1cat": w1cat,
        "W2ha": np.ascontiguousarray(np.vstack([W2ha1, W2ha1])),
        "W2hb": np.ascontiguousarray(W2hb),
        "bb": bb.reshape(H1, 1),
        "Wo": f(Wo).reshape(H1, 1),
        "bo": f(bo).reshape(1, 1),
    }
    in_maps = []
    for c in range(N_CORES):
        rows = slice(c * B, (c + 1) * B)
        stateT = np.ascontiguousarray(state[rows].T)  # [K, B]
        st_hi, st_lo = _hilo(stateT)
        shape = (N_SLAB, 128, ROWS_PER_PART * B)
        in_maps.append({
            "state_hi": st_hi.reshape(shape),
            "state_lo": st_lo.reshape(shape),
            "hiddenT": np.ascontiguousarray(hidden[rows].T),
            **common,
        })
    return in_maps


def unshard_outputs(results):
    hidden_new = np.concatenate([r["hnT"].T for r in results], axis=0)
    output = np.concatenate([r["outT"].reshape(B, 1) for r in results], axis=0)
    return output.astype(np.float32), hidden_new.astype(np.float32)


def run(in_maps, trace=False):
    res = run_bass_kernel_spmd(get_nc(), in_maps, list(range(N_CORES)),
                               trace=trace)
    _CACHE["last_results"] = res
    return res


def kernel(state, hidden, W1, b1, W2, b2, Wh, bh, Wo, bo):
    in_maps = shard_inputs(state, hidden, W1, b1, W2, b2, Wh, bh, Wo, bo)
    res = run(in_maps, trace=False)
    return unshard_outputs(res.results)


# revision 10
# speedup vs baseline: 1.2638x; 1.0612x over previous
"""Trainium2 Bass kernel for a small dense MLP head over a wide input.

Network (all f32, affine until the final sigmoid):
    level      = state @ W1 + b1                      # [B, 50]
    combined   = concat([level, hidden], axis=1)      # [B, 100]
    level2     = combined @ W2 + b2                   # [B, 32]
    hidden_new = level2 @ Wh + bh                     # [B, 50]
    output     = sigmoid(hidden_new @ Wo + bo)        # [B, 1]

Strategy: pure data parallel over the batch (8192 rows -> 1024 per core),
weights replicated.  Each shard's activations are passed transposed
([features, batch]) so the contraction dim sits on SBUF partitions — the
weights as stored ([in, out]) are then exactly the PE's lhsT layout and no
on-device transposes are needed anywhere.

The dominant cost is streaming state (16.8 MB/core).  The PE's native fp32
matmul lowers to 2 slow HW passes, so state and W1 are sent as an
exact-split bf16 pair (hi = bf16(x), lo = bf16(x - hi)) — the same
4 bytes/element of traffic.  All four product terms are computed in just
two bf16 matmuls per tile by packing [W1hi | W1lo] as one 100-column
stationary: the MM against state_hi yields hi@hi (psum partitions 0-49)
and lo@hi (50-99); the MM against state_lo adds hi@lo and lo@lo.  The sum
of the two partition halves is the exact product; it is never explicitly
merged — the next layer uses a duplicated stationary [W2a; W2a] and bias
[b1; 0] instead.  The concat with `hidden` also never materializes
(level2 = level @ W2[:50] + hidden @ W2[50:] as accumulating matmuls).

State is DMA'd in 1 MB contiguous slabs viewed as [128, 4096], which
row-permutes K — harmless because W1's rows are permuted identically (the
same natural reshape).  hi slabs ride the Sync HWDGE ring, lo slabs the
Scalar HWDGE ring, small constants the GpSimd SWDGE ring, so the two
big streams run on both hardware DGE rings in parallel.
"""

import sys

for _p in ("/opt/trn_rl_repo",):
    if _p not in sys.path:
        sys.path.insert(0, _p)

from contextlib import ExitStack

import ml_dtypes
import numpy as np

import concourse.bass as bass
import concourse.tile as tile
from concourse import bacc, mybir
from concourse.bass_utils import run_bass_kernel_spmd

N_CORES = 8
BATCH = 8192
B = BATCH // N_CORES  # 1024 rows per core
K = 4096              # state features (contraction dim of the big matmul)
H1 = 50               # level / hidden width
H2 = 32               # level2 width
HALF = 512            # one PSUM bank of f32; max moving free dim
SLAB_ROWS = 256       # K-rows per DMA slab -> [128, 2048] bf16 = 512 KB each
N_SLAB = K // SLAB_ROWS           # 16
ROWS_PER_PART = SLAB_ROWS // 128  # 4 K-rows interleaved per partition
N_CHUNK = N_SLAB * ROWS_PER_PART  # 32 K-chunks of 128

F32 = mybir.dt.float32
BF16 = mybir.dt.bfloat16
BF16_NP = ml_dtypes.bfloat16

_CACHE = {}


def _build_nc():
    """Build + compile the per-core Bass program (identical on all cores)."""
    nc = bacc.Bacc("TRN2", target_bir_lowering=False, debug=False)

    # ---- DRAM parameters (per-core shard views, pre-laid-out by the host) ----
    sthi_d = nc.dram_tensor("state_hi", [N_SLAB, 128, ROWS_PER_PART * B], BF16,
                            kind="ExternalInput")
    stlo_d = nc.dram_tensor("state_lo", [N_SLAB, 128, ROWS_PER_PART * B], BF16,
                            kind="ExternalInput")
    w1cat_d = nc.dram_tensor("W1cat", [128, N_CHUNK * 2 * H1], BF16,
                             kind="ExternalInput")
    hiddenT_d = nc.dram_tensor("hiddenT", [H1, B], F32, kind="ExternalInput")
    # Host-fused tail weights: W2ha = [W2a@Wh; W2a@Wh], W2hb = W2b@Wh,
    # bb = b1@W2a@Wh + b2@Wh + bh  (the W2->Wh chain is affine, so it folds).
    w2ha_d = nc.dram_tensor("W2ha", [2 * H1, H1], F32, kind="ExternalInput")
    w2hb_d = nc.dram_tensor("W2hb", [H1, H1], F32, kind="ExternalInput")
    bb_d = nc.dram_tensor("bb", [H1, 1], F32, kind="ExternalInput")
    wo_d = nc.dram_tensor("Wo", [H1, 1], F32, kind="ExternalInput")
    bo_d = nc.dram_tensor("bo", [1, 1], F32, kind="ExternalInput")
    hnT_d = nc.dram_tensor("hnT", [H1, B], F32, kind="ExternalOutput")
    outT_d = nc.dram_tensor("outT", [1, B], F32, kind="ExternalOutput")

    Ident = mybir.ActivationFunctionType.Identity
    Sigm = mybir.ActivationFunctionType.Sigmoid

    with tile.TileContext(nc) as tc, ExitStack() as ctx:
        consts = ctx.enter_context(tc.tile_pool(name="consts", bufs=1))
        stp = ctx.enter_context(tc.tile_pool(name="state", bufs=8))
        work = ctx.enter_context(tc.tile_pool(name="work", bufs=2))
        psum = ctx.enter_context(
            tc.tile_pool(name="psum", bufs=1, space=bass.MemorySpace.PSUM))

        # W1 leads the Scalar HWDGE ring (ahead of the state_lo slabs);
        # everything small goes via the GpSimd SWDGE ring.
        w1cat = consts.tile([128, N_CHUNK * 2 * H1], BF16, tag="w1cat")
        nc.scalar.dma_start(w1cat[:], w1cat_d[:])
        hidT = consts.tile([H1, B], F32, tag="hid")
        nc.gpsimd.dma_start(hidT[:], hiddenT_d[:])
        w2ha = consts.tile([2 * H1, H1], F32, tag="w2ha")
        nc.gpsimd.dma_start(w2ha[:], w2ha_d[:])
        w2hb = consts.tile([H1, H1], F32, tag="w2hb")
        nc.gpsimd.dma_start(w2hb[:], w2hb_d[:])
        wos = consts.tile([H1, 1], F32, tag="wo")
        nc.gpsimd.dma_start(wos[:], wo_d[:])
        bbs = consts.tile([H1, 1], F32, tag="bb")
        nc.gpsimd.dma_start(bbs[:], bb_d[:])
        bos = consts.tile([1, 1], F32, tag="bo")
        nc.gpsimd.dma_start(bos[:], bo_d[:])

        # Warm the ACT sigmoid table under the DMA shadow instead of in the
        # epilogue tail (~1.3 us table load on first use of a new func).
        sigwarm = consts.tile([1, 1], F32, tag="sigwarm")
        nc.scalar.activation(sigwarm[:], bos[:], Sigm, bias=bos[:])

        # ---- main accumulation over K ----
        # lv[h] psum [100, 512]: partitions 0-49 accumulate W1hi-terms,
        # 50-99 accumulate W1lo-terms; their sum is the exact f32 product.
        lv = [psum.tile([2 * H1, HALF], F32, tag=f"lv{h}", name=f"lv{h}")
              for h in range(B // HALF)]
        n_j = ROWS_PER_PART
        for s in range(N_SLAB):
            sthi = stp.tile([128, n_j * B], BF16, tag="sthi")
            nc.sync.dma_start(sthi[:], sthi_d[s])
            stlo = stp.tile([128, n_j * B], BF16, tag="stlo")
            nc.scalar.dma_start(stlo[:], stlo_d[s])
            for j in range(n_j):
                ch = s * n_j + j
                wc = w1cat[:, ch * 2 * H1:(ch + 1) * 2 * H1]
                for st, t in ((sthi, 0), (stlo, 1)):
                    for h in range(B // HALF):
                        nc.tensor.matmul(
                            lv[h][:],
                            wc,
                            st[:, j * B + h * HALF: j * B + h * HALF + HALF],
                            start=(s == 0 and j == 0 and t == 0),
                            stop=(s == N_SLAB - 1 and j == n_j - 1 and t == 1),
                        )

        # ---- epilogue per batch half: 2 fused matmuls + bias, then out head ----
        hns_tiles = []
        for h in range(B // HALF):
            cols = slice(h * HALF, (h + 1) * HALF)
            # pure PSUM->SBUF copy on the otherwise-idle DVE (bias folded
            # into bb host-side, so no ACT needed here)
            lvs = work.tile([2 * H1, HALF], F32, tag="lvs")
            nc.vector.tensor_copy(lvs[:], lv[h][:])

            hnp = psum.tile([H1, HALF], F32, tag="hn")
            nc.tensor.matmul(hnp[:], w2ha[:], lvs[:], start=True, stop=False)
            nc.tensor.matmul(hnp[:], w2hb[:], hidT[:, cols], start=False,
                             stop=True)
            hns = work.tile([H1, HALF], F32, tag="hns", name=f"hns{h}")
            nc.scalar.activation(hns[:], hnp[:], Ident, bias=bbs[:])
            nc.sync.dma_start(hnT_d[:, cols], hns[:])
            hns_tiles.append(hns)

        for h, hns in enumerate(hns_tiles):
            cols = slice(h * HALF, (h + 1) * HALF)
            op = psum.tile([1, HALF], F32, tag="op", name=f"op{h}")
            nc.tensor.matmul(op[:], wos[:], hns[:], start=True, stop=True)
            osb = work.tile([1, HALF], F32, tag="osb", name=f"osb{h}")
            nc.scalar.activation(osb[:], op[:], Sigm, bias=bos[:])
            nc.sync.dma_start(outT_d[:, cols], osb[:])

    nc.compile()
    return nc


def get_nc():
    if "nc" not in _CACHE:
        _CACHE["nc"] = _build_nc()
    return _CACHE["nc"]


def _hilo(x):
    hi = x.astype(BF16_NP)
    lo = (x - hi.astype(np.float32)).astype(BF16_NP)
    return hi, lo


def shard_inputs(state, hidden, W1, b1, W2, b2, Wh, bh, Wo, bo):
    """Full inputs -> one input dict per core, in the device layout."""
    f = lambda x: np.ascontiguousarray(np.asarray(x, dtype=np.float32))
    state, hidden = f(state), f(hidden)
    W1, W2, Wh = f(W1), f(W2), f(Wh)
    b1 = f(b1).reshape(H1, 1)
    # W1 rows permuted exactly like the state slab reshape: slab s holds K-rows
    # [512s, 512s+512), partition p within a slab holds rows 4p..4p+3.
    # Chunk ch covers K-rows with partition p -> row 128*ch' ... ; each chunk's
    # stationary is [hi | lo] side by side (100 columns).
    w1p = (W1.reshape(N_SLAB, 128, ROWS_PER_PART, H1).transpose(1, 0, 2, 3)
           .reshape(128, N_CHUNK, H1))
    w1hi, w1lo = _hilo(w1p)
    w1cat = np.ascontiguousarray(
        np.concatenate([w1hi, w1lo], axis=2).reshape(128, N_CHUNK * 2 * H1))
    # Fuse the affine W2 -> Wh chain (f64 on host for clean rounding):
    #   hidden_new = level @ (W2a@Wh) + hidden @ (W2b@Wh) + (b1@W2a@Wh + b2@Wh + bh)
    W2a64, W2b64 = np.float64(W2[:H1]), np.float64(W2[H1:])
    Wh64 = np.float64(Wh)
    W2ha1 = (W2a64 @ Wh64).astype(np.float32)  # [50, 50]
    W2hb = (W2b64 @ Wh64).astype(np.float32)   # [50, 50]
    bb = (np.float64(b1[:, 0]) @ W2a64 @ Wh64 + np.float64(f(b2)) @ Wh64
          + np.float64(f(bh))).astype(np.float32)
    common = {
        "W1cat": w1cat,
        "W2ha": np.ascontiguousarray(np.vstack([W2ha1, W2ha1])),
        "W2hb": np.ascontiguousarray(W2hb),
        "bb": bb.reshape(H1, 1),
        "Wo": f(Wo).reshape(H1, 1),
        "bo": f(bo).reshape(1, 1),
    }
    in_maps = []
    for c in range(N_CORES):
        rows = slice(c * B, (c + 1) * B)
        stateT = np.ascontiguousarray(state[rows].T)  # [K, B]
        st_hi, st_lo = _hilo(stateT)
        shape = (N_SLAB, 128, ROWS_PER_PART * B)
        in_maps.append({
            "state_hi": st_hi.reshape(shape),
            "state_lo": st_lo.reshape(shape),
            "hiddenT": np.ascontiguousarray(hidden[rows].T),
            **common,
        })
    return in_maps


def unshard_outputs(results):
    hidden_new = np.concatenate([r["hnT"].T for r in results], axis=0)
    output = np.concatenate([r["outT"].reshape(B, 1) for r in results], axis=0)
    return output.astype(np.float32), hidden_new.astype(np.float32)


def run(in_maps, trace=False):
    res = run_bass_kernel_spmd(get_nc(), in_maps, list(range(N_CORES)),
                               trace=trace)
    _CACHE["last_results"] = res
    return res


def kernel(state, hidden, W1, b1, W2, b2, Wh, bh, Wo, bo):
    in_maps = shard_inputs(state, hidden, W1, b1, W2, b2, Wh, bh, Wo, bo)
    res = run(in_maps, trace=False)
    return unshard_outputs(res.results)


# revision 19
# speedup vs baseline: 1.3305x; 1.0528x over previous
"""Trainium2 Bass kernel for a small dense MLP head over a wide input.

Network (all f32, affine until the final sigmoid):
    level      = state @ W1 + b1                      # [B, 50]
    combined   = concat([level, hidden], axis=1)      # [B, 100]
    level2     = combined @ W2 + b2                   # [B, 32]
    hidden_new = level2 @ Wh + bh                     # [B, 50]
    output     = sigmoid(hidden_new @ Wo + bo)        # [B, 1]

Strategy: pure data parallel over the batch (8192 rows -> 1024 per core),
weights replicated.  Each shard's activations are passed transposed
([features, batch]) so the contraction dim sits on SBUF partitions — the
weights as stored ([in, out]) are then exactly the PE's lhsT layout and no
on-device transposes are needed anywhere.

The dominant cost is streaming state (16.8 MB/core).  The PE's native fp32
matmul lowers to 2 slow HW passes, so state and W1 are sent as an
exact-split bf16 pair (hi = bf16(x), lo = bf16(x - hi)) — the same
4 bytes/element of traffic.  All four product terms are computed in just
two bf16 matmuls per tile by packing [W1hi | W1lo] as one 100-column
stationary: the MM against state_hi yields hi@hi (psum partitions 0-49)
and lo@hi (50-99); the MM against state_lo adds hi@lo and lo@lo.  The sum
of the two partition halves is the exact product; it is never explicitly
merged — the next layer uses a duplicated stationary [W2a; W2a] and bias
[b1; 0] instead.  The concat with `hidden` also never materializes
(level2 = level @ W2[:50] + hidden @ W2[50:] as accumulating matmuls).

State is DMA'd in 1 MB contiguous slabs viewed as [128, 4096], which
row-permutes K — harmless because W1's rows are permuted identically (the
same natural reshape).  hi slabs ride the Sync HWDGE ring, lo slabs the
Scalar HWDGE ring, small constants the GpSimd SWDGE ring, so the two
big streams run on both hardware DGE rings in parallel.
"""

import sys

for _p in ("/opt/trn_rl_repo",):
    if _p not in sys.path:
        sys.path.insert(0, _p)

from contextlib import ExitStack

import ml_dtypes
import numpy as np

import concourse.bass as bass
import concourse.tile as tile
from concourse import bacc, mybir
from concourse.bass_utils import run_bass_kernel_spmd

N_CORES = 8
BATCH = 8192
B = BATCH // N_CORES  # 1024 rows per core
K = 4096              # state features (contraction dim of the big matmul)
H1 = 50               # level / hidden width
H2 = 32               # level2 width
OUTP = 64             # partition of the fused pre-sigmoid output row (32-aligned)
HALF = 512            # one PSUM bank of f32; max moving free dim
SLAB_ROWS = 256       # K-rows per DMA slab -> [128, 2048] bf16 = 512 KB each
N_SLAB = K // SLAB_ROWS           # 16
ROWS_PER_PART = SLAB_ROWS // 128  # 4 K-rows interleaved per partition
N_CHUNK = N_SLAB * ROWS_PER_PART  # 32 K-chunks of 128

F32 = mybir.dt.float32
BF16 = mybir.dt.bfloat16
BF16_NP = ml_dtypes.bfloat16

_CACHE = {}


def _build_nc():
    """Build + compile the per-core Bass program (identical on all cores)."""
    nc = bacc.Bacc("TRN2", target_bir_lowering=False, debug=False)

    # ---- DRAM parameters (per-core shard views, pre-laid-out by the host) ----
    sthi_d = nc.dram_tensor("state_hi", [N_SLAB, 128, ROWS_PER_PART * B], BF16,
                            kind="ExternalInput")
    stlo_d = nc.dram_tensor("state_lo", [N_SLAB, 128, ROWS_PER_PART * B], BF16,
                            kind="ExternalInput")
    w1cat_d = nc.dram_tensor("W1cat", [128, N_CHUNK * 2 * H1], BF16,
                             kind="ExternalInput")
    hiddenT_d = nc.dram_tensor("hiddenT", [H1, B], F32, kind="ExternalInput")
    # Host-fused tail weights (the whole W2 -> Wh -> Wo chain is affine):
    #   W2hA = [[W2a@Wh; W2a@Wh] | [W2a@Wh@Wo; W2a@Wh@Wo]]   [100, 51]
    #   W2hB = [W2b@Wh | W2b@Wh@Wo]                           [50, 51]
    #   bb   = b1@W2a@Wh + b2@Wh + bh;   oc = bb@Wo + bo (scalar)
    # Column 50 of the shared stationary computes the pre-sigmoid output.
    w2ha_d = nc.dram_tensor("W2hA", [2 * H1, OUTP + 1], F32, kind="ExternalInput")
    w2hb_d = nc.dram_tensor("W2hB", [H1, OUTP + 1], F32, kind="ExternalInput")
    bb_d = nc.dram_tensor("bb", [H1, 1], F32, kind="ExternalInput")
    # oc lives on partition 50 so it is partition-aligned with the
    # pre-sigmoid output row of the hno psum tile
    oc_d = nc.dram_tensor("oc", [OUTP + 1, 1], F32, kind="ExternalInput")
    hnT_d = nc.dram_tensor("hnT", [H1, B], F32, kind="ExternalOutput")
    outT_d = nc.dram_tensor("outT", [1, B], F32, kind="ExternalOutput")

    Ident = mybir.ActivationFunctionType.Identity
    Sigm = mybir.ActivationFunctionType.Sigmoid

    with tile.TileContext(nc) as tc, ExitStack() as ctx:
        consts = ctx.enter_context(tc.tile_pool(name="consts", bufs=1))
        stp = ctx.enter_context(tc.tile_pool(name="state", bufs=8))
        work = ctx.enter_context(tc.tile_pool(name="work", bufs=2))
        psum = ctx.enter_context(
            tc.tile_pool(name="psum", bufs=1, space=bass.MemorySpace.PSUM))

        # W1 leads the Scalar HWDGE ring (ahead of the state_lo slabs);
        # everything small goes via the GpSimd SWDGE ring.
        w1cat = consts.tile([128, N_CHUNK * 2 * H1], BF16, tag="w1cat")
        nc.scalar.dma_start(w1cat[:], w1cat_d[:])
        hidT = consts.tile([H1, B], F32, tag="hid")
        nc.gpsimd.dma_start(hidT[:], hiddenT_d[:])
        w2ha = consts.tile([2 * H1, OUTP + 1], F32, tag="w2ha")
        nc.gpsimd.dma_start(w2ha[:], w2ha_d[:])
        w2hb = consts.tile([H1, OUTP + 1], F32, tag="w2hb")
        nc.gpsimd.dma_start(w2hb[:], w2hb_d[:])
        bbs = consts.tile([H1, 1], F32, tag="bb")
        nc.gpsimd.dma_start(bbs[:], bb_d[:])
        ocs = consts.tile([OUTP + 1, 1], F32, tag="oc")
        nc.gpsimd.dma_start(ocs[:], oc_d[:])

        # Warm the ACT sigmoid table under the DMA shadow instead of in the
        # epilogue tail (~1.3 us table load on first use of a new func).
        sigwarm = consts.tile([OUTP + 1, 1], F32, tag="sigwarm")
        nc.scalar.activation(sigwarm[OUTP:OUTP + 1, :], ocs[OUTP:OUTP + 1, :],
                             Sigm, bias=ocs[OUTP:OUTP + 1, :])

        # ---- main accumulation over K ----
        # lv[h] psum [100, 512]: partitions 0-49 accumulate W1hi-terms,
        # 50-99 accumulate W1lo-terms; their sum is the exact f32 product.
        lv = [psum.tile([2 * H1, HALF], F32, tag=f"lv{h}", name=f"lv{h}")
              for h in range(B // HALF)]
        # hno[h] accumulates [hidden_new.T ; pre-sigmoid out.T] (51 partitions).
        # The hidden-side matmul has no dependence on the state stream, so it
        # runs early, under the DMA shadow, as the start of the group.
        hno = [psum.tile([OUTP + 1, HALF], F32, tag=f"hno{h}", name=f"hno{h}")
               for h in range(B // HALF)]
        for h in range(B // HALF):
            nc.tensor.matmul(hno[h][:], w2hb[:],
                             hidT[:, h * HALF:(h + 1) * HALF],
                             start=True, stop=False)
        n_j = ROWS_PER_PART
        for s in range(N_SLAB):
            sthi = stp.tile([128, n_j * B], BF16, tag="sthi")
            nc.sync.dma_start(sthi[:], sthi_d[s])
            stlo = stp.tile([128, n_j * B], BF16, tag="stlo")
            nc.scalar.dma_start(stlo[:], stlo_d[s])
            for j in range(n_j):
                ch = s * n_j + j
                wc = w1cat[:, ch * 2 * H1:(ch + 1) * 2 * H1]
                for st, t in ((sthi, 0), (stlo, 1)):
                    for h in range(B // HALF):
                        nc.tensor.matmul(
                            lv[h][:],
                            wc,
                            st[:, j * B + h * HALF: j * B + h * HALF + HALF],
                            start=(s == 0 and j == 0 and t == 0),
                            stop=(s == N_SLAB - 1 and j == n_j - 1 and t == 1),
                        )

        # ---- epilogue per batch half: one fused matmul closes the group ----
        for h in range(B // HALF):
            cols = slice(h * HALF, (h + 1) * HALF)
            # pure PSUM->SBUF copy on the otherwise-idle DVE (bias folded
            # into bb host-side, so no ACT needed here)
            lvs = work.tile([2 * H1, HALF], F32, tag="lvs")
            nc.vector.tensor_copy(lvs[:], lv[h][:])

            nc.tensor.matmul(hno[h][:], w2ha[:], lvs[:], start=False, stop=True)
            hns = work.tile([H1, HALF], F32, tag="hns", name=f"hns{h}")
            nc.scalar.activation(hns[:], hno[h][0:H1, :], Ident, bias=bbs[:])
            nc.sync.dma_start(hnT_d[:, cols], hns[:])
            # partition 50 holds the pre-sigmoid output row; keep the sbuf
            # tile at the same base partition so in/out stay aligned
            osb = work.tile([OUTP + 1, HALF], F32, tag="osb", name=f"osb{h}")
            nc.scalar.activation(osb[OUTP:OUTP + 1, :], hno[h][OUTP:OUTP + 1, :],
                                 Sigm, bias=ocs[OUTP:OUTP + 1, :])
            nc.sync.dma_start(outT_d[:, cols], osb[OUTP:OUTP + 1, :])

    nc.compile()
    return nc


def get_nc():
    if "nc" not in _CACHE:
        _CACHE["nc"] = _build_nc()
    return _CACHE["nc"]


def _hilo(x):
    hi = x.astype(BF16_NP)
    lo = (x - hi.astype(np.float32)).astype(BF16_NP)
    return hi, lo


def shard_inputs(state, hidden, W1, b1, W2, b2, Wh, bh, Wo, bo):
    """Full inputs -> one input dict per core, in the device layout."""
    f = lambda x: np.ascontiguousarray(np.asarray(x, dtype=np.float32))
    state, hidden = f(state), f(hidden)
    W1, W2, Wh = f(W1), f(W2), f(Wh)
    b1 = f(b1).reshape(H1, 1)
    # W1 rows permuted exactly like the state slab reshape: slab s holds K-rows
    # [512s, 512s+512), partition p within a slab holds rows 4p..4p+3.
    # Chunk ch covers K-rows with partition p -> row 128*ch' ... ; each chunk's
    # stationary is [hi | lo] side by side (100 columns).
    w1p = (W1.reshape(N_SLAB, 128, ROWS_PER_PART, H1).transpose(1, 0, 2, 3)
           .reshape(128, N_CHUNK, H1))
    w1hi, w1lo = _hilo(w1p)
    w1cat = np.ascontiguousarray(
        np.concatenate([w1hi, w1lo], axis=2).reshape(128, N_CHUNK * 2 * H1))
    # Fuse the affine W2 -> Wh -> Wo chain (f64 on host for clean rounding):
    #   hidden_new = level @ (W2a@Wh) + hidden @ (W2b@Wh) + bb
    #   output     = sigmoid(level @ (W2a@Wh@Wo) + hidden @ (W2b@Wh@Wo) + oc)
    W2a64, W2b64 = np.float64(W2[:H1]), np.float64(W2[H1:])
    Wh64 = np.float64(Wh)
    Wo64 = np.float64(f(Wo).reshape(H1, 1))
    W2ha1 = W2a64 @ Wh64                      # [50, 50]
    W2hb1 = W2b64 @ Wh64                      # [50, 50]
    bb = (np.float64(b1[:, 0]) @ W2a64 @ Wh64 + np.float64(f(b2)) @ Wh64
          + np.float64(f(bh)))                # [50]
    oc_val = float(bb @ Wo64[:, 0] + float(np.asarray(bo).reshape(-1)[0]))
    pad = np.zeros((H1, OUTP - H1), np.float64)
    w2hA = np.hstack([W2ha1, pad, W2ha1 @ Wo64]).astype(np.float32)  # [50, 65]
    w2hB = np.hstack([W2hb1, pad, W2hb1 @ Wo64]).astype(np.float32)  # [50, 65]
    oc = np.zeros((OUTP + 1, 1), np.float32)
    oc[OUTP, 0] = oc_val
    common = {
        "W1cat": w1cat,
        "W2hA": np.ascontiguousarray(np.vstack([w2hA, w2hA])),
        "W2hB": np.ascontiguousarray(w2hB),
        "bb": bb.astype(np.float32).reshape(H1, 1),
        "oc": oc,
    }
    in_maps = []
    for c in range(N_CORES):
        rows = slice(c * B, (c + 1) * B)
        stateT = np.ascontiguousarray(state[rows].T)  # [K, B]
        st_hi, st_lo = _hilo(stateT)
        shape = (N_SLAB, 128, ROWS_PER_PART * B)
        in_maps.append({
            "state_hi": st_hi.reshape(shape),
            "state_lo": st_lo.reshape(shape),
            "hiddenT": np.ascontiguousarray(hidden[rows].T),
            **common,
        })
    return in_maps


def unshard_outputs(results):
    hidden_new = np.concatenate([r["hnT"].T for r in results], axis=0)
    output = np.concatenate([r["outT"].reshape(B, 1) for r in results], axis=0)
    return output.astype(np.float32), hidden_new.astype(np.float32)


def run(in_maps, trace=False):
    res = run_bass_kernel_spmd(get_nc(), in_maps, list(range(N_CORES)),
                               trace=trace)
    _CACHE["last_results"] = res
    return res


def kernel(state, hidden, W1, b1, W2, b2, Wh, bh, Wo, bo):
    in_maps = shard_inputs(state, hidden, W1, b1, W2, b2, Wh, bh, Wo, bo)
    res = run(in_maps, trace=False)
    return unshard_outputs(res.results)
